# revision 40
# baseline (speedup 1.0000x reference)
"""Trainium2 Bass kernel for AsyncFeatureExtraction (segment_reduce).

v11:
  - channel mask DMA'd (off the DVE); stX multiplies it in place
  - xall built in [P, C, NCH] layout against a materialized crow24
    const so every DVE build op has contiguous 16-bit inner dims (2x)
  - step masks via PE (ramp + -ceil(t) matmuls) + ACT Sign, giving
    +-1 steps; the (s+1)/2 correction is folded into a single ones
    matmul and a 0.5 scale baked into sqrt/occ-copy scales
  - |D| folded into the min-reduce (apply_absolute_value) from PSUM
  - dw = sqrt(ivd)/2 via ACT Sqrt scale (kernel_scale is 0.5)
  - PE warmup lifts the HAM clock gate during the DMA window

Math (per batch, 1 batch per core):
  * rank[n] = # earlier same-channel points, via segmented cumsum scan;
    cross-segment prefix via chm matmul; rank extracted by 6 f32
    matmuls (lhsT=maskg block, rhs=segsel) -> [128, 24] chunk-major.
  * grid routing: grid += rkoh_c.T @ [t_hi|t_lo|occ|v] (bf16 exact).
  * inv_density per 4-channel group: one matmul builds s_j - s_i from
    transposed bf16 hi/lo planes + BIG*I eye matmul; min over |D|.
  * Z/cnt/V/ZT1 cumulative step-histograms from +-1 sign-steps;
    out = w3.T @ [S1R;ZR;VR]^T.
"""

import os
import numpy as np

B, N, T, C, D, CO = 8, 3072, 128, 32, 8, 64
P = 128
NCH = N // P          # 24 chunks of 128 consecutive points
NB = 6                # 128-point blocks per segment
NSEG = 4
SEGN = N // NSEG      # 768
NG = 8                # channel groups
CG = C // NG          # 4 channels per group
BIG = 1e10

_cache = {}

# packed f32 const layout (small, DMA'd first so the scan starts early)
_OFF = {}
_cw = 0
for _name, _w in [
    ("segsel", NSEG), ("blin", 1), ("imp", 1), ("pmp", 1), ("iota", 1),
]:
    _OFF[_name] = (_cw, _w)
    _cw += _w
CW = _cw

# packed bf16 const layout
_OFFB = {}
_cwb = 0
for _name, _w in [("idb", P), ("w3b", CO), ("irowb", P), ("iotab", 1)]:
    _OFFB[_name] = (_cwb, _w)
    _cwb += _w
CWB = _cwb


def _build_nc():
    from contextlib import ExitStack

    import concourse.bass as bass
    import concourse.tile as tile
    from concourse import bacc, mybir

    f32 = mybir.dt.float32
    bf16 = mybir.dt.bfloat16
    i32 = mybir.dt.int32
    ALU = mybir.AluOpType
    ACT = mybir.ActivationFunctionType
    AX = mybir.AxisListType

    nc = bacc.Bacc(None)

    xF = nc.declare_dram_parameter("xF", [1, N], bf16, isOutput=False)
    xP = nc.declare_dram_parameter("xP", [3, N], f32, isOutput=False)
    cst = nc.declare_dram_parameter("cst", [P, CW], f32, isOutput=False)
    chm = nc.declare_dram_parameter("chm", [P, P], f32, isOutput=False)
    cstb = nc.declare_dram_parameter("cstb", [P, CWB], bf16, isOutput=False)
    mskb = nc.declare_dram_parameter("mskb", [32, C * P], bf16, isOutput=False)
    xC = nc.declare_dram_parameter("xC", [1, C * NCH], bf16, isOutput=False)
    out_ext = nc.declare_dram_parameter("out", [CO, T], f32, isOutput=True)

    def dram_ap(handle, offset, pattern):
        return bass.AP(handle[:].tensor, offset, pattern)

    with tile.TileContext(nc) as tc, ExitStack() as ctx:
        const = ctx.enter_context(tc.tile_pool(name="const", bufs=1))
        pp = ctx.enter_context(tc.tile_pool(name="perpoint", bufs=1))
        rk = ctx.enter_context(tc.tile_pool(name="rank", bufs=1))
        big = ctx.enter_context(tc.tile_pool(name="big", bufs=1))
        gr = ctx.enter_context(tc.tile_pool(name="grid", bufs=1))
        band = ctx.enter_context(tc.tile_pool(name="band", bufs=1))
        sb = ctx.enter_context(tc.tile_pool(name="stageD", bufs=1))
        psum = ctx.enter_context(tc.tile_pool(name="psum", bufs=1, space="PSUM"))

        # ---- DMAs (bandwidth is shared by all 8 cores; early = small) ----
        f_seg = rk.tile([P, SEGN], bf16)
        for s in range(NSEG):
            nc.sync.dma_start(
                f_seg[32 * s : 32 * s + 32, :],
                xF[0][SEGN * s : SEGN * (s + 1)][None, :].to_broadcast([32, SEGN]),
            )
        # xP is host-pretransposed so the chunk-major load is contiguous:
        # xP[k, p*NCH + ch] = x[128*ch + p, k]
        pv = pp.tile([P, 3, NCH], f32)
        nc.sync.dma_start(pv[:], dram_ap(xP, 0, [[NCH, P], [N, 3], [1, NCH]]))

        cst_t = const.tile([P, CW], f32)
        nc.scalar.dma_start(cst_t[:], cst[:])
        cstb_t = const.tile([P, CWB], bf16)
        nc.scalar.dma_start(cstb_t[:], cstb[:])
        chm_tt = const.tile([P, P], f32)
        nc.scalar.dma_start(chm_tt[:], chm[:])
        crow24 = big.tile([P, C, NCH], bf16)
        nc.gpsimd.dma_start(
            crow24[:].rearrange("p c h -> p (c h)"),
            xC[0][None, :].to_broadcast([P, C * NCH]),
        )
        rhsAll = big.tile([P, C, P], bf16)
        rhs3 = rhsAll[:]
        msk_src = mskb[:].rearrange("p (c j) -> p c j", c=C)
        nc.gpsimd.dma_start(rhs3[0:32], msk_src)
        nc.gpsimd.dma_start(rhs3[32:64], rhs3[0:32])
        nc.gpsimd.dma_start(rhs3[64:128], rhs3[0:64])

        def cslice(name, rows=P):
            o, w = _OFF[name]
            return cst_t[0:rows, o : o + w]

        def cbslice(name, rows=P):
            o, w = _OFFB[name]
            return cstb_t[0:rows, o : o + w]

        chm_t = chm_tt[:]
        segsel_t = cslice("segsel")
        blin_c = cslice("blin", CO)
        imp_c = cslice("imp")
        pmp_c = cslice("pmp")
        iota_c = cslice("iota")
        id_b = cbslice("idb")
        w3b = cbslice("w3b", 96)
        irow_b = cbslice("irowb")
        iota_b = cbslice("iotab")

        bige4 = const.tile([P, CG, P], bf16)
        bigeye4 = bige4[:].rearrange("p a b -> p (a b)")

        # PE warmup as the first tensor work: ~3.5us of sustained matmul
        # activity flips the HAM clock gate to 2.4 GHz before the serial
        # rank matmuls; later PE work keeps it warm.
        warm_p = psum.tile([P, P], f32, tag="tpose")
        for _ in range(20):
            nc.tensor.matmul(warm_p[:], lhsT=id_b, rhs=id_b, start=True, stop=True)

        f_t = pv[:, 0, :]
        v_t = pv[:, 1, :]
        t_t = pv[:, 2, :]

        # ---- ranks: segmented scan + 6 seg-reduce matmuls ----
        oh_seg = rk.tile([P, SEGN], f32)
        nc.vector.tensor_scalar(oh_seg[:], f_seg[:], iota_c, None, ALU.is_equal)
        zseg = rk.tile([P, SEGN], f32)
        nc.gpsimd.memset(zseg[:], 0.0)
        csum = rk.tile([P, SEGN], f32)
        nc.vector.tensor_tensor_scan(
            csum[:], oh_seg[:], zseg[:], 0.0, op0=ALU.add, op1=ALU.add
        )
        totals = rk.tile([P, 1], f32)
        nc.vector.tensor_copy(totals[:], csum[:, SEGN - 1 : SEGN])
        a_p = psum.tile([P, 1], f32, tag="scratch")
        nc.tensor.matmul(a_p[:], lhsT=chm_t, rhs=totals[:], start=True, stop=True)
        a_s = rk.tile([P, 1], f32)
        nc.vector.tensor_scalar(a_s[:], a_p[:], -1.0, None, ALU.add)
        csum2 = rk.tile([P, SEGN], f32)
        maskg = rk.tile([P, SEGN], f32)
        rank_b = pp.tile([P, NCH], bf16)
        # BIG*I eye blocks for the all-pairs diagonal kill (issued after
        # the scan so it doesn't head-block the vector queue)
        nc.vector.tensor_scalar(
            bige4[:], id_b.unsqueeze(1).to_broadcast([P, CG, P]), BIG, None,
            ALU.mult,
        )
        # rank_p[p, s, bk] = rank of point n = 768*s + 128*bk + p; flat
        # (s, bk) order matches pv's contiguous chunk index n//128.
        rank_p = psum.tile([P, NSEG, NB], f32, tag="scratch")
        for h in range(2):
            sl = slice(h * 384, h * 384 + 384)
            nc.vector.tensor_scalar(
                csum2[:, sl], csum[:, sl], a_s[:, 0:1], None, ALU.add
            )
            nc.vector.tensor_tensor(
                maskg[:, sl], csum2[:, sl], oh_seg[:, sl], op=ALU.mult
            )
            for bk in range(3 * h, 3 * h + 3):
                nc.tensor.matmul(
                    rank_p[:, :, bk],
                    lhsT=maskg[:, P * bk : P * bk + P],
                    rhs=segsel_t,
                    start=True,
                    stop=True,
                )
            # ranks < 3072: bf16 rounds >=256 away from 0..127, never to
            # a slot index, so the bf16 compare below stays exact.
            nc.scalar.activation(
                rank_b[:].rearrange("p (s b) -> p s b", s=NSEG)[:, :, 3 * h : 3 * h + 3],
                rank_p[:, :, 3 * h : 3 * h + 3],
                ACT.Copy,
            )

        # ---- per-point planes (bf16 so DVE builds run at 2x) ----
        thi_t = pp.tile([P, NCH], bf16)
        nc.scalar.activation(thi_t[:], t_t, ACT.Copy)
        thi_f = pp.tile([P, NCH], f32)
        nc.scalar.activation(thi_f[:], thi_t[:], ACT.Copy)
        tlo_t = pp.tile([P, NCH], f32)
        nc.vector.tensor_tensor(tlo_t[:], t_t, thi_f[:], op=ALU.subtract)
        tlo_b = pp.tile([P, NCH], bf16)
        nc.vector.tensor_copy(tlo_b[:], tlo_t[:])
        # per-point ceil(t) (exact for trunc or round f32->i32 casts)
        tcp_i = pp.tile([P, NCH], i32)
        nc.vector.tensor_copy(tcp_i[:], t_t)
        tcp_f = pp.tile([P, NCH], f32)
        nc.vector.tensor_copy(tcp_f[:], tcp_i[:])
        tcp_g = pp.tile([P, NCH], f32)
        nc.vector.tensor_tensor(tcp_g[:], t_t, tcp_f[:], op=ALU.is_gt)
        nc.vector.tensor_tensor(tcp_f[:], tcp_f[:], tcp_g[:], op=ALU.add)
        v_b = pp.tile([P, NCH], bf16)
        nc.vector.tensor_copy(v_b[:], v_t)
        f_b = pp.tile([P, NCH], bf16)
        nc.vector.tensor_copy(f_b[:], f_t)

        # ---- channel-routed value planes [P, C, NCH] + rank one-hots ----
        xall = big.tile([P, 5, C, NCH], bf16)
        nc.vector.tensor_tensor(
            xall[:, 2, :, :], crow24[:],
            f_b[:].unsqueeze(1).to_broadcast([P, C, NCH]), op=ALU.is_equal,
        )
        oh3 = xall[:, 2, :, :]
        nc.vector.tensor_tensor(
            xall[:, 0, :, :], oh3,
            thi_t[:].unsqueeze(1).to_broadcast([P, C, NCH]), op=ALU.mult,
        )
        nc.vector.tensor_tensor(
            xall[:, 1, :, :], oh3,
            tlo_b[:].unsqueeze(1).to_broadcast([P, C, NCH]), op=ALU.mult,
        )
        nc.vector.tensor_tensor(
            xall[:, 3, :, :], oh3,
            v_b[:].unsqueeze(1).to_broadcast([P, C, NCH]), op=ALU.mult,
        )
        nc.vector.tensor_tensor(
            xall[:, 4, :, :], oh3,
            tcp_f[:].unsqueeze(1).to_broadcast([P, C, NCH]), op=ALU.mult,
        )

        rkoh = big.tile([P, NSEG, NB, P], bf16)
        rank_b4 = rank_b[:].rearrange("p (s b) -> p s b", s=NSEG)
        grid_p = psum.tile([P, 5, C], f32, tag="grid")
        nmm = 0
        for h in range(2):
            bsl = slice(3 * h, 3 * h + 3)
            nc.vector.tensor_tensor(
                rkoh[:, :, bsl, :],
                rank_b4[:, :, bsl].unsqueeze(3).to_broadcast([P, NSEG, 3, P]),
                irow_b.unsqueeze(1).unsqueeze(1).to_broadcast([P, NSEG, 3, P]),
                op=ALU.is_equal,
            )
            for s in range(NSEG):
                for bk in range(3 * h, 3 * h + 3):
                    ch = NB * s + bk
                    nc.tensor.matmul(
                        grid_p[:], lhsT=rkoh[:, s, bk, :], rhs=xall[:, :, :, ch],
                        start=(nmm == 0), stop=(nmm == NCH - 1),
                    )
                    nmm += 1

        # ---- grid extraction ----
        thi_g = gr.tile([P, C], f32)
        nc.scalar.activation(thi_g[:], grid_p[:, 0, :], ACT.Copy)
        t_g = gr.tile([P, C], f32)
        nc.vector.tensor_tensor(t_g[:], thi_g[:], grid_p[:, 1, :], op=ALU.add)
        occ_g = gr.tile([P, C], f32)
        nc.scalar.activation(occ_g[:], grid_p[:, 2, :], ACT.Copy)
        v_g = gr.tile([P, C], f32)
        nc.scalar.activation(v_g[:], grid_p[:, 3, :], ACT.Copy)
        s_g = gr.tile([P, C], f32)
        nc.vector.tensor_scalar(s_g[:], occ_g[:], BIG, -BIG, ALU.mult, op1=ALU.add)
        nc.vector.tensor_tensor(s_g[:], s_g[:], t_g[:], op=ALU.add)

        # ---- s hi/lo planes, transposed (+negated) via one matmul ----
        s_lo = gr.tile([P, C], f32)
        shiloX = gr.tile([P, P], bf16)
        nc.vector.tensor_copy(shiloX[:, 0:32], s_g[:])
        nc.vector.tensor_tensor(s_lo[:], s_g[:], shiloX[:, 0:32], op=ALU.subtract)
        nc.vector.tensor_copy(shiloX[:, 32:64], s_lo[:])
        nc.vector.tensor_scalar(shiloX[:, 64:96], s_g[:], -1.0, None, ALU.mult)
        nc.vector.tensor_scalar(shiloX[:, 96:128], s_lo[:], -1.0, None, ALU.mult)
        stpX = psum.tile([P, P], f32, tag="tpose")
        nc.tensor.matmul(stpX[:], lhsT=shiloX[:], rhs=id_b, start=True, stop=True)

        # steps[p, c, tau] = (tau >= ceil(t)); ceil came through the grid
        tc_b = gr.tile([P, C], bf16)
        nc.vector.tensor_copy(tc_b[:], grid_p[:, 4, :])
        steps = big.tile([P, C, P], bf16)
        for h in range(2):
            sl = slice(h * 16, h * 16 + 16)
            nc.vector.tensor_tensor(
                steps[:, sl, :],
                irow_b.unsqueeze(1).to_broadcast([P, 16, P]),
                tc_b[:, sl].unsqueeze(2).to_broadcast([P, 16, P]),
                op=ALU.is_ge,
            )

        st2 = gr.tile([64, P], bf16)
        nc.vector.tensor_copy(st2[:], stpX[0:64, :])
        onesneg = gr.tile([P, P], bf16)
        nc.gpsimd.memset(onesneg[0:64, :], 1.0)
        nc.vector.tensor_copy(onesneg[64:128, :], stpX[64:128, :])

        # stX = mask * st2 (in place, top half of rhsAll)
        for h in range(2):
            sl = slice(h * 16, h * 16 + 16)
            nc.vector.tensor_tensor(
                rhs3[0:64, sl, :], rhs3[0:64, sl, :],
                st2[:].unsqueeze(1).to_broadcast([64, 16, P]), op=ALU.mult,
            )

        # ---- all-pairs min + sign-steps + dw + weights + histogram ----
        ivd = gr.tile([P, C], f32)
        dw = gr.tile([P, C], f32)
        w2f = gr.tile([P, C], f32)
        wN = gr.tile([P, 4, C], bf16)
        nc.scalar.activation(wN[:, 0, :], occ_g[:], ACT.Copy)
        hist_p = psum.tile([P, C, 4], f32, tag="hist")
        for g in range(NG):
            gs = slice(g * CG, g * CG + CG)
            sgb = psum.tile([P, CG, P], f32, tag=f"sgb{g % 2}", bufs=2)
            sgb_flat = sgb[:].rearrange("p a b -> p (a b)")
            nc.tensor.matmul(
                sgb_flat, lhsT=onesneg[:],
                rhs=rhs3[:, gs, :].rearrange("p a b -> p (a b)"),
                start=True, stop=False, skip_group_check=True,
            )
            nc.tensor.matmul(
                sgb_flat, lhsT=id_b, rhs=bigeye4,
                start=False, stop=True, skip_group_check=True,
            )
            dbuf = band.tile([P, CG, P], bf16, tag="dbf", bufs=2)
            nc.scalar.activation(dbuf[:], sgb[:], ACT.Abs)
            nc.vector.tensor_reduce(ivd[:, gs], dbuf[:], axis=AX.X, op=ALU.min)
            if g % 4 == 3:
                hh = g // 4
                hs = slice(hh * 16, hh * 16 + 16)
                nc.vector.tensor_scalar(dw[:, hs], ivd[:, hs], 2.0**-11, None, ALU.max)
                nc.scalar.activation(dw[:, hs], dw[:, hs], ACT.Sqrt)
                nc.vector.tensor_tensor(w2f[:, hs], occ_g[:, hs], dw[:, hs], op=ALU.mult)
                nc.vector.tensor_copy(wN[:, 1, hs], w2f[:, hs])
                nc.vector.tensor_tensor(wN[:, 2, hs], w2f[:, hs], v_g[:, hs], op=ALU.mult)
                nc.vector.tensor_tensor(wN[:, 3, hs], w2f[:, hs], t_g[:, hs], op=ALU.mult)
                for ch in range(hh * 16, hh * 16 + 16):
                    nc.tensor.matmul(
                        hist_p[:, ch, :], lhsT=steps[:, ch, :],
                        rhs=wN[:, :, ch], start=True, stop=True,
                    )

        # ---- combine (tau on partitions) ----
        cnt_v = hist_p[:, :, 0]
        z_v = hist_p[:, :, 1]
        v_v = hist_p[:, :, 2]
        zt1_v = hist_p[:, :, 3]

        r_t = sb.tile([P, C], f32)
        ce_t = sb.tile([P, C], f32)
        nc.vector.tensor_scalar(r_t[:], z_v, 1e-10, None, ALU.add)
        nc.vector.tensor_scalar(ce_t[:], cnt_v, 1e-10, None, ALU.add)
        nc.vector.tensor_tensor(r_t[:], r_t[:], ce_t[:], op=ALU.mult)
        nc.vector.reciprocal(r_t[:], r_t[:])
        s1_t = sb.tile([P, C], f32)
        nc.vector.tensor_scalar(s1_t[:], zt1_v, imp_c, None, ALU.mult)
        zp_t = sb.tile([P, C], f32)
        nc.vector.tensor_scalar(zp_t[:], z_v, pmp_c, None, ALU.mult)
        nc.vector.tensor_tensor(s1_t[:], s1_t[:], zp_t[:], op=ALU.subtract)

        pack = sb.tile([P, 96], bf16)
        nc.vector.tensor_tensor(pack[:, 0:32], s1_t[:], r_t[:], op=ALU.mult)
        nc.vector.tensor_tensor(pack[:, 32:64], z_v, r_t[:], op=ALU.mult)
        nc.vector.tensor_tensor(pack[:, 64:96], v_v, r_t[:], op=ALU.mult)

        packT_p = psum.tile([96, P], f32, tag="tpose")
        nc.tensor.matmul(packT_p[:], lhsT=pack[:], rhs=id_b, start=True, stop=True)
        packT = sb.tile([96, P], bf16)
        nc.vector.tensor_copy(packT[:], packT_p[:])
        out_p = psum.tile([CO, T], f32, tag="tpose")
        nc.tensor.matmul(out_p[:], lhsT=w3b, rhs=packT[:], start=True, stop=True)
        out_t = sb.tile([CO, T], f32)
        nc.vector.tensor_scalar(out_t[:], out_p[:], blin_c, None, ALU.add)
        nc.sync.dma_start(out_ext[:], out_t[:])

    nc.compile()
    return nc


def _prep_inputs(x, out_positions, W_dist, b_dist, emb, W_vals, b_vals, W_lin, b_lin, kernel_scale):
    import ml_dtypes

    bf = ml_dtypes.bfloat16
    assert abs(float(kernel_scale) - 0.5) < 1e-6  # dw = sqrt(ivd) baked in
    x = np.asarray(x, np.float32)
    pos = np.asarray(out_positions, np.float32)
    max_pos = float(pos.max())
    Wl = np.asarray(W_lin, np.float32).reshape(CO, C, D)
    emb2 = np.asarray(emb, np.float32)[:C] + np.asarray(b_dist, np.float32) + np.asarray(
        b_vals, np.float32
    )
    wd2 = (Wl * np.asarray(W_dist, np.float32)).sum(-1).T
    we2 = np.einsum("ocd,cd->oc", Wl, emb2).T
    wv2 = (Wl * np.asarray(W_vals, np.float32)).sum(-1).T

    q = np.arange(P)
    seg_sel = ((q // 32)[:, None] == np.arange(NSEG)[None, :]).astype(np.float32)
    chm_m = (
        ((q % C)[:, None] == (q % C)[None, :])
        & ((q // C)[:, None] < (q // C)[None, :])
    ).astype(np.float32)

    cst = np.zeros((P, CW), np.float32)

    def put(name, arr, rows=P):
        o, w = _OFF[name]
        cst[0:rows, o : o + w] = arr

    put("segsel", seg_sel)
    put("blin", np.asarray(b_lin, np.float32)[:, None], CO)
    put("imp", np.full((P, 1), 1.0 / max_pos, np.float32))
    put("pmp", (pos / max_pos)[:, None])
    put("iota", (q % 32).astype(np.float32)[:, None])

    cstb = np.zeros((P, CWB), np.float32)

    def putb(name, arr, rows=P):
        o, w = _OFFB[name]
        cstb[0:rows, o : o + w] = arr

    putb("idb", np.eye(P, dtype=np.float32))
    w3 = np.concatenate([wd2, we2, wv2], axis=0)  # (96, CO)
    putb("w3b", w3.astype(np.float32), 96)
    putb("irowb", np.tile(np.arange(P, dtype=np.float32), (P, 1)))
    putb("iotab", (q % 32).astype(np.float32)[:, None])
    cstb = cstb.astype(bf)

    msk = ((q % 32)[0:32, None] == np.arange(C)[None, :]).astype(np.float32)
    mskb = np.ascontiguousarray(
        np.repeat(msk[:, :, None], P, axis=2).reshape(32, C * P).astype(bf)
    )
    xC_a = np.repeat(np.arange(C, dtype=np.float32), NCH)[None, :].astype(bf)

    in_maps = []
    for b in range(B):
        xTb = np.ascontiguousarray(x[b].T)
        xPb = np.ascontiguousarray(
            xTb.reshape(3, NCH, P).transpose(0, 2, 1).reshape(3, N)
        )
        xFb = np.ascontiguousarray(xTb[0:1]).astype(bf)
        in_maps.append(
            {"xF": xFb, "xP": xPb, "cst": cst, "cstb": cstb, "mskb": mskb,
             "xC": np.ascontiguousarray(xC_a), "chm": chm_m}
        )
    return in_maps


def kernel(**inputs) -> np.ndarray:
    from concourse.bass_utils import run_bass_kernel_spmd

    if "nc" not in _cache:
        _cache["nc"] = _build_nc()
    nc = _cache["nc"]

    in_maps = _prep_inputs(**inputs)
    res = run_bass_kernel_spmd(
        nc, in_maps, core_ids=list(range(B)),
        trace=bool(int(os.environ.get("KERNEL_TRACE", "0"))),
    )
    if res.exec_time_ns is not None:
        _cache["exec_time_ns"] = res.exec_time_ns
        _cache["last_result"] = res
    out = np.stack([res.results[i]["out"] for i in range(B)]).astype(np.float32)
    return out


# revision 41
# speedup vs baseline: 1.0356x; 1.0356x over previous
"""Trainium2 Bass kernel for AsyncFeatureExtraction (segment_reduce).

v11:
  - channel mask DMA'd (off the DVE); stX multiplies it in place
  - xall built in [P, C, NCH] layout against a materialized crow24
    const so every DVE build op has contiguous 16-bit inner dims (2x)
  - step masks via PE (ramp + -ceil(t) matmuls) + ACT Sign, giving
    +-1 steps; the (s+1)/2 correction is folded into a single ones
    matmul and a 0.5 scale baked into sqrt/occ-copy scales
  - |D| folded into the min-reduce (apply_absolute_value) from PSUM
  - dw = sqrt(ivd)/2 via ACT Sqrt scale (kernel_scale is 0.5)
  - PE warmup lifts the HAM clock gate during the DMA window

Math (per batch, 1 batch per core):
  * rank[n] = # earlier same-channel points, via segmented cumsum scan;
    cross-segment prefix via chm matmul; rank extracted by 6 f32
    matmuls (lhsT=maskg block, rhs=segsel) -> [128, 24] chunk-major.
  * grid routing: grid += rkoh_c.T @ [t_hi|t_lo|occ|v] (bf16 exact).
  * inv_density per 4-channel group: one matmul builds s_j - s_i from
    transposed bf16 hi/lo planes + BIG*I eye matmul; min over |D|.
  * Z/cnt/V/ZT1 cumulative step-histograms from +-1 sign-steps;
    out = w3.T @ [S1R;ZR;VR]^T.
"""

import os
import numpy as np

B, N, T, C, D, CO = 8, 3072, 128, 32, 8, 64
P = 128
NCH = N // P          # 24 chunks of 128 consecutive points
NB = 6                # 128-point blocks per segment
NSEG = 4
SEGN = N // NSEG      # 768
NG = 8                # channel groups
CG = C // NG          # 4 channels per group
BIG = 1e10

_cache = {}

# packed f32 const layout (small, DMA'd first so the scan starts early)
_OFF = {}
_cw = 0
for _name, _w in [
    ("segsel", NSEG), ("blin", 1), ("imp", 1), ("pmp", 1), ("iota", 1),
]:
    _OFF[_name] = (_cw, _w)
    _cw += _w
CW = _cw

# packed bf16 const layout
_OFFB = {}
_cwb = 0
for _name, _w in [("idb", P), ("w3b", CO), ("irowb", P), ("iotab", 1)]:
    _OFFB[_name] = (_cwb, _w)
    _cwb += _w
CWB = _cwb


def _build_nc():
    from contextlib import ExitStack

    import concourse.bass as bass
    import concourse.tile as tile
    from concourse import bacc, mybir

    f32 = mybir.dt.float32
    bf16 = mybir.dt.bfloat16
    i32 = mybir.dt.int32
    ALU = mybir.AluOpType
    ACT = mybir.ActivationFunctionType
    AX = mybir.AxisListType

    nc = bacc.Bacc(None)

    xF = nc.declare_dram_parameter("xF", [1, N], bf16, isOutput=False)
    xP = nc.declare_dram_parameter("xP", [3, N], f32, isOutput=False)
    cst = nc.declare_dram_parameter("cst", [P, CW], f32, isOutput=False)
    chm = nc.declare_dram_parameter("chm", [P, P], f32, isOutput=False)
    cstb = nc.declare_dram_parameter("cstb", [P, CWB], bf16, isOutput=False)
    mskb = nc.declare_dram_parameter("mskb", [32, C * P], bf16, isOutput=False)
    xC = nc.declare_dram_parameter("xC", [1, C * NCH], bf16, isOutput=False)
    out_ext = nc.declare_dram_parameter("out", [CO, T], f32, isOutput=True)

    def dram_ap(handle, offset, pattern):
        return bass.AP(handle[:].tensor, offset, pattern)

    with tile.TileContext(nc) as tc, ExitStack() as ctx:
        const = ctx.enter_context(tc.tile_pool(name="const", bufs=1))
        pp = ctx.enter_context(tc.tile_pool(name="perpoint", bufs=1))
        rk = ctx.enter_context(tc.tile_pool(name="rank", bufs=1))
        big = ctx.enter_context(tc.tile_pool(name="big", bufs=1))
        gr = ctx.enter_context(tc.tile_pool(name="grid", bufs=1))
        band = ctx.enter_context(tc.tile_pool(name="band", bufs=1))
        sb = ctx.enter_context(tc.tile_pool(name="stageD", bufs=1))
        psum = ctx.enter_context(tc.tile_pool(name="psum", bufs=1, space="PSUM"))

        # ---- DMAs (bandwidth is shared by all 8 cores; early = small) ----
        f_seg = rk.tile([P, SEGN], bf16)
        for s in range(NSEG):
            nc.sync.dma_start(
                f_seg[32 * s : 32 * s + 32, :],
                xF[0][SEGN * s : SEGN * (s + 1)][None, :].to_broadcast([32, SEGN]),
            )
        # xP is host-pretransposed so the chunk-major load is contiguous:
        # xP[k, p*NCH + ch] = x[128*ch + p, k]
        pv = pp.tile([P, 3, NCH], f32)
        nc.sync.dma_start(pv[:], dram_ap(xP, 0, [[NCH, P], [N, 3], [1, NCH]]))

        cst_t = const.tile([P, CW], f32)
        nc.scalar.dma_start(cst_t[:], cst[:])
        cstb_t = const.tile([P, CWB], bf16)
        nc.scalar.dma_start(cstb_t[:], cstb[:])
        chm_tt = const.tile([P, P], f32)
        nc.scalar.dma_start(chm_tt[:], chm[:])
        crow24 = big.tile([P, C, NCH], bf16)
        nc.gpsimd.dma_start(
            crow24[:].rearrange("p c h -> p (c h)"),
            xC[0][None, :].to_broadcast([P, C * NCH]),
        )
        rhsAll = big.tile([P, C, P], bf16)
        rhs3 = rhsAll[:]
        msk_src = mskb[:].rearrange("p (c j) -> p c j", c=C)
        nc.gpsimd.dma_start(rhs3[0:32], msk_src)
        nc.gpsimd.dma_start(rhs3[32:64], rhs3[0:32])
        nc.gpsimd.dma_start(rhs3[64:128], rhs3[0:64])

        def cslice(name, rows=P):
            o, w = _OFF[name]
            return cst_t[0:rows, o : o + w]

        def cbslice(name, rows=P):
            o, w = _OFFB[name]
            return cstb_t[0:rows, o : o + w]

        chm_t = chm_tt[:]
        segsel_t = cslice("segsel")
        blin_c = cslice("blin", CO)
        imp_c = cslice("imp")
        pmp_c = cslice("pmp")
        iota_c = cslice("iota")
        id_b = cbslice("idb")
        w3b = cbslice("w3b", 96)
        irow_b = cbslice("irowb")
        iota_b = cbslice("iotab")

        bige4 = const.tile([P, CG, P], bf16)
        bigeye4 = bige4[:].rearrange("p a b -> p (a b)")

        # PE warmup as the first tensor work: ~3.5us of sustained matmul
        # activity flips the HAM clock gate to 2.4 GHz before the serial
        # rank matmuls; later PE work keeps it warm.
        warm_p = psum.tile([P, P], f32, tag="tpose")
        for _ in range(36):
            nc.tensor.matmul(warm_p[:], lhsT=id_b, rhs=id_b, start=True, stop=True)

        f_t = pv[:, 0, :]
        v_t = pv[:, 1, :]
        t_t = pv[:, 2, :]

        # ---- ranks: segmented scan + 6 seg-reduce matmuls ----
        oh_seg = rk.tile([P, SEGN], f32)
        nc.vector.tensor_scalar(oh_seg[:], f_seg[:], iota_c, None, ALU.is_equal)
        zseg = rk.tile([P, SEGN], f32)
        nc.gpsimd.memset(zseg[:], 0.0)
        csum = rk.tile([P, SEGN], f32)
        nc.vector.tensor_tensor_scan(
            csum[:], oh_seg[:], zseg[:], 0.0, op0=ALU.add, op1=ALU.add
        )
        totals = rk.tile([P, 1], f32)
        nc.vector.tensor_copy(totals[:], csum[:, SEGN - 1 : SEGN])
        a_p = psum.tile([P, 1], f32, tag="scratch")
        nc.tensor.matmul(a_p[:], lhsT=chm_t, rhs=totals[:], start=True, stop=True)
        a_s = rk.tile([P, 1], f32)
        nc.vector.tensor_scalar(a_s[:], a_p[:], -1.0, None, ALU.add)
        csum2 = rk.tile([P, SEGN], f32)
        maskg = rk.tile([P, SEGN], f32)
        rank_b = pp.tile([P, NCH], bf16)
        # BIG*I eye blocks for the all-pairs diagonal kill (issued after
        # the scan so it doesn't head-block the vector queue)
        nc.vector.tensor_scalar(
            bige4[:], id_b.unsqueeze(1).to_broadcast([P, CG, P]), BIG, None,
            ALU.mult,
        )
        # rank_p[p, s, bk] = rank of point n = 768*s + 128*bk + p; flat
        # (s, bk) order matches pv's contiguous chunk index n//128.
        rank_p = psum.tile([P, NSEG, NB], f32, tag="scratch")
        for h in range(2):
            sl = slice(h * 384, h * 384 + 384)
            nc.vector.tensor_scalar(
                csum2[:, sl], csum[:, sl], a_s[:, 0:1], None, ALU.add
            )
            nc.vector.tensor_tensor(
                maskg[:, sl], csum2[:, sl], oh_seg[:, sl], op=ALU.mult
            )
            for bk in range(3 * h, 3 * h + 3):
                nc.tensor.matmul(
                    rank_p[:, :, bk],
                    lhsT=maskg[:, P * bk : P * bk + P],
                    rhs=segsel_t,
                    start=True,
                    stop=True,
                )
            # ranks < 3072: bf16 rounds >=256 away from 0..127, never to
            # a slot index, so the bf16 compare below stays exact.
            nc.scalar.activation(
                rank_b[:].rearrange("p (s b) -> p s b", s=NSEG)[:, :, 3 * h : 3 * h + 3],
                rank_p[:, :, 3 * h : 3 * h + 3],
                ACT.Copy,
            )

        # ---- per-point planes (bf16 so DVE builds run at 2x) ----
        thi_t = pp.tile([P, NCH], bf16)
        nc.scalar.activation(thi_t[:], t_t, ACT.Copy)
        thi_f = pp.tile([P, NCH], f32)
        nc.scalar.activation(thi_f[:], thi_t[:], ACT.Copy)
        tlo_t = pp.tile([P, NCH], f32)
        nc.vector.tensor_tensor(tlo_t[:], t_t, thi_f[:], op=ALU.subtract)
        tlo_b = pp.tile([P, NCH], bf16)
        nc.vector.tensor_copy(tlo_b[:], tlo_t[:])
        # per-point ceil(t) (exact for trunc or round f32->i32 casts)
        tcp_i = pp.tile([P, NCH], i32)
        nc.vector.tensor_copy(tcp_i[:], t_t)
        tcp_f = pp.tile([P, NCH], f32)
        nc.vector.tensor_copy(tcp_f[:], tcp_i[:])
        tcp_g = pp.tile([P, NCH], f32)
        nc.vector.tensor_tensor(tcp_g[:], t_t, tcp_f[:], op=ALU.is_gt)
        nc.vector.tensor_tensor(tcp_f[:], tcp_f[:], tcp_g[:], op=ALU.add)
        v_b = pp.tile([P, NCH], bf16)
        nc.vector.tensor_copy(v_b[:], v_t)
        f_b = pp.tile([P, NCH], bf16)
        nc.vector.tensor_copy(f_b[:], f_t)

        # ---- channel-routed value planes [P, C, NCH] + rank one-hots ----
        xall = big.tile([P, 5, C, NCH], bf16)
        nc.vector.tensor_tensor(
            xall[:, 2, :, :], crow24[:],
            f_b[:].unsqueeze(1).to_broadcast([P, C, NCH]), op=ALU.is_equal,
        )
        oh3 = xall[:, 2, :, :]
        nc.vector.tensor_tensor(
            xall[:, 0, :, :], oh3,
            thi_t[:].unsqueeze(1).to_broadcast([P, C, NCH]), op=ALU.mult,
        )
        nc.vector.tensor_tensor(
            xall[:, 1, :, :], oh3,
            tlo_b[:].unsqueeze(1).to_broadcast([P, C, NCH]), op=ALU.mult,
        )
        nc.vector.tensor_tensor(
            xall[:, 3, :, :], oh3,
            v_b[:].unsqueeze(1).to_broadcast([P, C, NCH]), op=ALU.mult,
        )
        nc.vector.tensor_tensor(
            xall[:, 4, :, :], oh3,
            tcp_f[:].unsqueeze(1).to_broadcast([P, C, NCH]), op=ALU.mult,
        )

        rkoh = big.tile([P, NSEG, NB, P], bf16)
        rank_b4 = rank_b[:].rearrange("p (s b) -> p s b", s=NSEG)
        grid_p = psum.tile([P, 5, C], f32, tag="grid")
        nmm = 0
        for h in range(2):
            bsl = slice(3 * h, 3 * h + 3)
            nc.vector.tensor_tensor(
                rkoh[:, :, bsl, :],
                rank_b4[:, :, bsl].unsqueeze(3).to_broadcast([P, NSEG, 3, P]),
                irow_b.unsqueeze(1).unsqueeze(1).to_broadcast([P, NSEG, 3, P]),
                op=ALU.is_equal,
            )
            for s in range(NSEG):
                for bk in range(3 * h, 3 * h + 3):
                    ch = NB * s + bk
                    nc.tensor.matmul(
                        grid_p[:], lhsT=rkoh[:, s, bk, :], rhs=xall[:, :, :, ch],
                        start=(nmm == 0), stop=(nmm == NCH - 1),
                    )
                    nmm += 1

        # ---- grid extraction ----
        thi_g = gr.tile([P, C], f32)
        nc.scalar.activation(thi_g[:], grid_p[:, 0, :], ACT.Copy)
        t_g = gr.tile([P, C], f32)
        nc.vector.tensor_tensor(t_g[:], thi_g[:], grid_p[:, 1, :], op=ALU.add)
        occ_g = gr.tile([P, C], f32)
        nc.scalar.activation(occ_g[:], grid_p[:, 2, :], ACT.Copy)
        v_g = gr.tile([P, C], f32)
        nc.scalar.activation(v_g[:], grid_p[:, 3, :], ACT.Copy)
        s_g = gr.tile([P, C], f32)
        nc.vector.tensor_scalar(s_g[:], occ_g[:], BIG, -BIG, ALU.mult, op1=ALU.add)
        nc.vector.tensor_tensor(s_g[:], s_g[:], t_g[:], op=ALU.add)

        # ---- s hi/lo planes, transposed (+negated) via one matmul ----
        s_lo = gr.tile([P, C], f32)
        shiloX = gr.tile([P, P], bf16)
        nc.vector.tensor_copy(shiloX[:, 0:32], s_g[:])
        nc.vector.tensor_tensor(s_lo[:], s_g[:], shiloX[:, 0:32], op=ALU.subtract)
        nc.vector.tensor_copy(shiloX[:, 32:64], s_lo[:])
        nc.vector.tensor_scalar(shiloX[:, 64:96], s_g[:], -1.0, None, ALU.mult)
        nc.vector.tensor_scalar(shiloX[:, 96:128], s_lo[:], -1.0, None, ALU.mult)
        stpX = psum.tile([P, P], f32, tag="tpose")
        nc.tensor.matmul(stpX[:], lhsT=shiloX[:], rhs=id_b, start=True, stop=True)

        # steps[p, c, tau] = (tau >= ceil(t)); ceil came through the grid
        tc_b = gr.tile([P, C], bf16)
        nc.vector.tensor_copy(tc_b[:], grid_p[:, 4, :])
        steps = big.tile([P, C, P], bf16)
        for h in range(2):
            sl = slice(h * 16, h * 16 + 16)
            nc.vector.tensor_tensor(
                steps[:, sl, :],
                irow_b.unsqueeze(1).to_broadcast([P, 16, P]),
                tc_b[:, sl].unsqueeze(2).to_broadcast([P, 16, P]),
                op=ALU.is_ge,
            )

        st2 = gr.tile([64, P], bf16)
        nc.vector.tensor_copy(st2[:], stpX[0:64, :])
        onesneg = gr.tile([P, P], bf16)
        nc.gpsimd.memset(onesneg[0:64, :], 1.0)
        nc.vector.tensor_copy(onesneg[64:128, :], stpX[64:128, :])

        # stX = mask * st2 (in place, top half of rhsAll)
        for h in range(2):
            sl = slice(h * 16, h * 16 + 16)
            nc.vector.tensor_tensor(
                rhs3[0:64, sl, :], rhs3[0:64, sl, :],
                st2[:].unsqueeze(1).to_broadcast([64, 16, P]), op=ALU.mult,
            )

        # ---- all-pairs min + sign-steps + dw + weights + histogram ----
        ivd = gr.tile([P, C], f32)
        dw = gr.tile([P, C], f32)
        w2f = gr.tile([P, C], f32)
        wN = gr.tile([P, 4, C], bf16)
        nc.scalar.activation(wN[:, 0, :], occ_g[:], ACT.Copy)
        hist_p = psum.tile([P, C, 4], f32, tag="hist")
        for g in range(NG):
            gs = slice(g * CG, g * CG + CG)
            sgb = psum.tile([P, CG, P], f32, tag=f"sgb{g % 2}", bufs=2)
            sgb_flat = sgb[:].rearrange("p a b -> p (a b)")
            nc.tensor.matmul(
                sgb_flat, lhsT=onesneg[:],
                rhs=rhs3[:, gs, :].rearrange("p a b -> p (a b)"),
                start=True, stop=False, skip_group_check=True,
            )
            nc.tensor.matmul(
                sgb_flat, lhsT=id_b, rhs=bigeye4,
                start=False, stop=True, skip_group_check=True,
            )
            nc.vector.tensor_reduce(ivd[:, gs], sgb[:], axis=AX.X, op=ALU.min,
                                    apply_absolute_value=True)
            if g % 4 == 3:
                hh = g // 4
                hs = slice(hh * 16, hh * 16 + 16)
                nc.vector.tensor_scalar(dw[:, hs], ivd[:, hs], 2.0**-11, None, ALU.max)
                nc.scalar.activation(dw[:, hs], dw[:, hs], ACT.Sqrt)
                nc.vector.tensor_tensor(w2f[:, hs], occ_g[:, hs], dw[:, hs], op=ALU.mult)
                nc.vector.tensor_copy(wN[:, 1, hs], w2f[:, hs])
                nc.vector.tensor_tensor(wN[:, 2, hs], w2f[:, hs], v_g[:, hs], op=ALU.mult)
                nc.vector.tensor_tensor(wN[:, 3, hs], w2f[:, hs], t_g[:, hs], op=ALU.mult)
                for ch in range(hh * 16, hh * 16 + 16):
                    nc.tensor.matmul(
                        hist_p[:, ch, :], lhsT=steps[:, ch, :],
                        rhs=wN[:, :, ch], start=True, stop=True,
                    )

        # ---- combine (tau on partitions) ----
        cnt_v = hist_p[:, :, 0]
        z_v = hist_p[:, :, 1]
        v_v = hist_p[:, :, 2]
        zt1_v = hist_p[:, :, 3]

        r_t = sb.tile([P, C], f32)
        ce_t = sb.tile([P, C], f32)
        nc.vector.tensor_scalar(r_t[:], z_v, 1e-10, None, ALU.add)
        nc.vector.tensor_scalar(ce_t[:], cnt_v, 1e-10, None, ALU.add)
        nc.vector.tensor_tensor(r_t[:], r_t[:], ce_t[:], op=ALU.mult)
        nc.vector.reciprocal(r_t[:], r_t[:])
        s1_t = sb.tile([P, C], f32)
        nc.vector.tensor_scalar(s1_t[:], zt1_v, imp_c, None, ALU.mult)
        zp_t = sb.tile([P, C], f32)
        nc.vector.tensor_scalar(zp_t[:], z_v, pmp_c, None, ALU.mult)
        nc.vector.tensor_tensor(s1_t[:], s1_t[:], zp_t[:], op=ALU.subtract)

        pack = sb.tile([P, 96], bf16)
        nc.vector.tensor_tensor(pack[:, 0:32], s1_t[:], r_t[:], op=ALU.mult)
        nc.vector.tensor_tensor(pack[:, 32:64], z_v, r_t[:], op=ALU.mult)
        nc.vector.tensor_tensor(pack[:, 64:96], v_v, r_t[:], op=ALU.mult)

        packT_p = psum.tile([96, P], f32, tag="tpose")
        nc.tensor.matmul(packT_p[:], lhsT=pack[:], rhs=id_b, start=True, stop=True)
        packT = sb.tile([96, P], bf16)
        nc.vector.tensor_copy(packT[:], packT_p[:])
        out_p = psum.tile([CO, T], f32, tag="tpose")
        nc.tensor.matmul(out_p[:], lhsT=w3b, rhs=packT[:], start=True, stop=True)
        out_t = sb.tile([CO, T], f32)
        nc.vector.tensor_scalar(out_t[:], out_p[:], blin_c, None, ALU.add)
        nc.sync.dma_start(out_ext[:], out_t[:])

    nc.compile()
    return nc


def _prep_inputs(x, out_positions, W_dist, b_dist, emb, W_vals, b_vals, W_lin, b_lin, kernel_scale):
    import ml_dtypes

    bf = ml_dtypes.bfloat16
    assert abs(float(kernel_scale) - 0.5) < 1e-6  # dw = sqrt(ivd) baked in
    x = np.asarray(x, np.float32)
    pos = np.asarray(out_positions, np.float32)
    max_pos = float(pos.max())
    Wl = np.asarray(W_lin, np.float32).reshape(CO, C, D)
    emb2 = np.asarray(emb, np.float32)[:C] + np.asarray(b_dist, np.float32) + np.asarray(
        b_vals, np.float32
    )
    wd2 = (Wl * np.asarray(W_dist, np.float32)).sum(-1).T
    we2 = np.einsum("ocd,cd->oc", Wl, emb2).T
    wv2 = (Wl * np.asarray(W_vals, np.float32)).sum(-1).T

    q = np.arange(P)
    seg_sel = ((q // 32)[:, None] == np.arange(NSEG)[None, :]).astype(np.float32)
    chm_m = (
        ((q % C)[:, None] == (q % C)[None, :])
        & ((q // C)[:, None] < (q // C)[None, :])
    ).astype(np.float32)

    cst = np.zeros((P, CW), np.float32)

    def put(name, arr, rows=P):
        o, w = _OFF[name]
        cst[0:rows, o : o + w] = arr

    put("segsel", seg_sel)
    put("blin", np.asarray(b_lin, np.float32)[:, None], CO)
    put("imp", np.full((P, 1), 1.0 / max_pos, np.float32))
    put("pmp", (pos / max_pos)[:, None])
    put("iota", (q % 32).astype(np.float32)[:, None])

    cstb = np.zeros((P, CWB), np.float32)

    def putb(name, arr, rows=P):
        o, w = _OFFB[name]
        cstb[0:rows, o : o + w] = arr

    putb("idb", np.eye(P, dtype=np.float32))
    w3 = np.concatenate([wd2, we2, wv2], axis=0)  # (96, CO)
    putb("w3b", w3.astype(np.float32), 96)
    putb("irowb", np.tile(np.arange(P, dtype=np.float32), (P, 1)))
    putb("iotab", (q % 32).astype(np.float32)[:, None])
    cstb = cstb.astype(bf)

    msk = ((q % 32)[0:32, None] == np.arange(C)[None, :]).astype(np.float32)
    mskb = np.ascontiguousarray(
        np.repeat(msk[:, :, None], P, axis=2).reshape(32, C * P).astype(bf)
    )
    xC_a = np.repeat(np.arange(C, dtype=np.float32), NCH)[None, :].astype(bf)

    in_maps = []
    for b in range(B):
        xTb = np.ascontiguousarray(x[b].T)
        xPb = np.ascontiguousarray(
            xTb.reshape(3, NCH, P).transpose(0, 2, 1).reshape(3, N)
        )
        xFb = np.ascontiguousarray(xTb[0:1]).astype(bf)
        in_maps.append(
            {"xF": xFb, "xP": xPb, "cst": cst, "cstb": cstb, "mskb": mskb,
             "xC": np.ascontiguousarray(xC_a), "chm": chm_m}
        )
    return in_maps


def kernel(**inputs) -> np.ndarray:
    from concourse.bass_utils import run_bass_kernel_spmd

    if "nc" not in _cache:
        _cache["nc"] = _build_nc()
    nc = _cache["nc"]

    in_maps = _prep_inputs(**inputs)
    res = run_bass_kernel_spmd(
        nc, in_maps, core_ids=list(range(B)),
        trace=bool(int(os.environ.get("KERNEL_TRACE", "0"))),
    )
    if res.exec_time_ns is not None:
        _cache["exec_time_ns"] = res.exec_time_ns
        _cache["last_result"] = res
    out = np.stack([res.results[i]["out"] for i in range(B)]).astype(np.float32)
    return out


# revision 42
# speedup vs baseline: 1.0461x; 1.0102x over previous
"""Trainium2 Bass kernel for AsyncFeatureExtraction (segment_reduce).

Final design (one batch per core, 8 cores, pure batch data-parallelism):
  * rank[n] = # earlier same-channel points via a segmented cumsum scan
    (4 segments x 32 channels on partitions); cross-segment prefix via a
    chm matmul; ranks extracted straight into chunk-major [128, 24]
    layout by 6 f32 matmuls (lhsT = maskg block, rhs = segsel).
  * grid routing: per 128-point chunk, grid += rkoh.T @ [thi|tlo|occ|v|
    ceil(t)] with bf16-exact hi/lo time planes; one-hots built by DVE
    broadcast compares with all-bf16 operands (2x rate), against
    materialized crow/irow bf16 consts.
  * inv_density: per 4-channel group one bf16 matmul forms s_j - s_i
    from transposed +-[hi|lo] planes ([ones;-st] stationary x
    [mask*st; mask] moving) + a BIG*I eye matmul for the diagonal;
    ivd = min-reduce with apply_absolute_value; dw = sqrt(ivd)
    (kernel_scale is 0.5 for this problem).
  * Z/cnt/V/ZT1 as cumulative step-histograms: steps = (tau >=
    ceil(t)) built bf16-exact (ceil scattered through the grid);
    one [128x128]x[128x4] matmul per channel.
  * combine: R = 1/((Z+eps)(cnt+eps)); out = w3.T @ [S1R;ZR;VR]^T with
    host-prefolded weights; bias add; single output DMA.

Perf notes baked in:
  - all DVE build ops use 16-bit operands with contiguous inner dims
    where possible (any f32 operand halves DVE throughput)
  - DMA bandwidth is shared by all 8 cores: constants are minimized,
    split so early-needed bytes land first, and spread across the
    SP/ACT/Pool queues; the channel mask is one 256KB load plus
    on-chip SBUF->SBUF clones; x is sent twice (n-order for the scan
    broadcast, host-pretransposed chunk-major for the point planes)
  - a PE warmup burst lifts the HAM clock gate during the DMA window
  - PSUM: 8 banks exactly (scratch, grid, tpose, sgb0 x2, sgb1 x2, hist)
"""

import os
import numpy as np

B, N, T, C, D, CO = 8, 3072, 128, 32, 8, 64
P = 128
NCH = N // P          # 24 chunks of 128 consecutive points
NB = 6                # 128-point blocks per segment
NSEG = 4
SEGN = N // NSEG      # 768
NG = 8                # channel groups
CG = C // NG          # 4 channels per group
BIG = 1e10

_cache = {}

# packed f32 const layout (small, DMA'd first so the scan starts early)
_OFF = {}
_cw = 0
for _name, _w in [
    ("segsel", NSEG), ("blin", 1), ("imp", 1), ("pmp", 1), ("iota", 1),
]:
    _OFF[_name] = (_cw, _w)
    _cw += _w
CW = _cw

# packed bf16 const layout
_OFFB = {}
_cwb = 0
for _name, _w in [("idb", P), ("w3b", CO), ("irowb", P), ("iotab", 1)]:
    _OFFB[_name] = (_cwb, _w)
    _cwb += _w
CWB = _cwb


def _build_nc():
    from contextlib import ExitStack

    import concourse.bass as bass
    import concourse.tile as tile
    from concourse import bacc, mybir

    f32 = mybir.dt.float32
    bf16 = mybir.dt.bfloat16
    i32 = mybir.dt.int32
    ALU = mybir.AluOpType
    ACT = mybir.ActivationFunctionType
    AX = mybir.AxisListType

    nc = bacc.Bacc(None)

    xF = nc.declare_dram_parameter("xF", [1, N], bf16, isOutput=False)
    xP = nc.declare_dram_parameter("xP", [3, N], f32, isOutput=False)
    cst = nc.declare_dram_parameter("cst", [P, CW], f32, isOutput=False)
    chm = nc.declare_dram_parameter("chm", [P, P], f32, isOutput=False)
    cstb = nc.declare_dram_parameter("cstb", [P, CWB], bf16, isOutput=False)
    mskb = nc.declare_dram_parameter("mskb", [32, C * P], bf16, isOutput=False)
    xC = nc.declare_dram_parameter("xC", [1, C * NCH], bf16, isOutput=False)
    out_ext = nc.declare_dram_parameter("out", [CO, T], f32, isOutput=True)

    def dram_ap(handle, offset, pattern):
        return bass.AP(handle[:].tensor, offset, pattern)

    with tile.TileContext(nc) as tc, ExitStack() as ctx:
        const = ctx.enter_context(tc.tile_pool(name="const", bufs=1))
        pp = ctx.enter_context(tc.tile_pool(name="perpoint", bufs=1))
        rk = ctx.enter_context(tc.tile_pool(name="rank", bufs=1))
        big = ctx.enter_context(tc.tile_pool(name="big", bufs=1))
        gr = ctx.enter_context(tc.tile_pool(name="grid", bufs=1))
        sb = ctx.enter_context(tc.tile_pool(name="stageD", bufs=1))
        psum = ctx.enter_context(tc.tile_pool(name="psum", bufs=1, space="PSUM"))

        # ---- DMAs (bandwidth is shared by all 8 cores; early = small) ----
        f_seg = rk.tile([P, SEGN], bf16)
        for s in range(NSEG):
            nc.sync.dma_start(
                f_seg[32 * s : 32 * s + 32, :],
                xF[0][SEGN * s : SEGN * (s + 1)][None, :].to_broadcast([32, SEGN]),
            )
        # xP is host-pretransposed so the chunk-major load is contiguous:
        # xP[k, p*NCH + ch] = x[128*ch + p, k]
        pv = pp.tile([P, 3, NCH], f32)
        nc.sync.dma_start(pv[:], dram_ap(xP, 0, [[NCH, P], [N, 3], [1, NCH]]))

        cst_t = const.tile([P, CW], f32)
        nc.scalar.dma_start(cst_t[:], cst[:])
        cstb_t = const.tile([P, CWB], bf16)
        nc.scalar.dma_start(cstb_t[:], cstb[:])
        chm_tt = const.tile([P, P], f32)
        nc.scalar.dma_start(chm_tt[:], chm[:])
        crow24 = big.tile([P, C, NCH], bf16)
        nc.gpsimd.dma_start(
            crow24[:].rearrange("p c h -> p (c h)"),
            xC[0][None, :].to_broadcast([P, C * NCH]),
        )
        rhsAll = big.tile([P, C, P], bf16)
        rhs3 = rhsAll[:]
        msk_src = mskb[:].rearrange("p (c j) -> p c j", c=C)
        nc.gpsimd.dma_start(rhs3[0:32], msk_src)
        nc.gpsimd.dma_start(rhs3[32:64], rhs3[0:32])
        nc.gpsimd.dma_start(rhs3[64:128], rhs3[0:64])

        def cslice(name, rows=P):
            o, w = _OFF[name]
            return cst_t[0:rows, o : o + w]

        def cbslice(name, rows=P):
            o, w = _OFFB[name]
            return cstb_t[0:rows, o : o + w]

        chm_t = chm_tt[:]
        segsel_t = cslice("segsel")
        blin_c = cslice("blin", CO)
        imp_c = cslice("imp")
        pmp_c = cslice("pmp")
        iota_c = cslice("iota")
        id_b = cbslice("idb")
        w3b = cbslice("w3b", 96)
        irow_b = cbslice("irowb")

        bige4 = const.tile([P, CG, P], bf16)
        bigeye4 = bige4[:].rearrange("p a b -> p (a b)")

        # PE warmup as the first tensor work: ~3.5us of sustained matmul
        # activity flips the HAM clock gate to 2.4 GHz before the serial
        # rank matmuls; later PE work keeps it warm.
        warm_p = psum.tile([P, P], f32, tag="tpose")
        for _ in range(36):
            nc.tensor.matmul(warm_p[:], lhsT=id_b, rhs=id_b, start=True, stop=True)

        f_t = pv[:, 0, :]
        v_t = pv[:, 1, :]
        t_t = pv[:, 2, :]

        # ---- ranks: segmented scan + 6 seg-reduce matmuls ----
        oh_seg = rk.tile([P, SEGN], f32)
        nc.vector.tensor_scalar(oh_seg[:], f_seg[:], iota_c, None, ALU.is_equal)
        zseg = rk.tile([P, SEGN], f32)
        nc.gpsimd.memset(zseg[:], 0.0)
        csum = rk.tile([P, SEGN], f32)
        nc.vector.tensor_tensor_scan(
            csum[:], oh_seg[:], zseg[:], 0.0, op0=ALU.add, op1=ALU.add
        )
        totals = rk.tile([P, 1], f32)
        nc.vector.tensor_copy(totals[:], csum[:, SEGN - 1 : SEGN])
        a_p = psum.tile([P, 1], f32, tag="scratch")
        nc.tensor.matmul(a_p[:], lhsT=chm_t, rhs=totals[:], start=True, stop=True)
        a_s = rk.tile([P, 1], f32)
        nc.vector.tensor_scalar(a_s[:], a_p[:], -1.0, None, ALU.add)
        csum2 = rk.tile([P, SEGN], f32)
        maskg = rk.tile([P, SEGN], f32)
        rank_b = pp.tile([P, NCH], bf16)
        # BIG*I eye blocks for the all-pairs diagonal kill (issued after
        # the scan so it doesn't head-block the vector queue)
        nc.vector.tensor_scalar(
            bige4[:], id_b.unsqueeze(1).to_broadcast([P, CG, P]), BIG, None,
            ALU.mult,
        )
        # rank_p[p, s, bk] = rank of point n = 768*s + 128*bk + p; flat
        # (s, bk) order matches pv's contiguous chunk index n//128.
        rank_p = psum.tile([P, NSEG, NB], f32, tag="scratch")
        for h in range(2):
            sl = slice(h * 384, h * 384 + 384)
            nc.vector.tensor_scalar(
                csum2[:, sl], csum[:, sl], a_s[:, 0:1], None, ALU.add
            )
            nc.vector.tensor_tensor(
                maskg[:, sl], csum2[:, sl], oh_seg[:, sl], op=ALU.mult
            )
            for bk in range(3 * h, 3 * h + 3):
                nc.tensor.matmul(
                    rank_p[:, :, bk],
                    lhsT=maskg[:, P * bk : P * bk + P],
                    rhs=segsel_t,
                    start=True,
                    stop=True,
                )
            # ranks < 3072: bf16 rounds >=256 away from 0..127, never to
            # a slot index, so the bf16 compare below stays exact.
            nc.scalar.activation(
                rank_b[:].rearrange("p (s b) -> p s b", s=NSEG)[:, :, 3 * h : 3 * h + 3],
                rank_p[:, :, 3 * h : 3 * h + 3],
                ACT.Copy,
            )

        # ---- per-point planes (bf16 so DVE builds run at 2x) ----
        thi_t = pp.tile([P, NCH], bf16)
        nc.scalar.activation(thi_t[:], t_t, ACT.Copy)
        thi_f = pp.tile([P, NCH], f32)
        nc.scalar.activation(thi_f[:], thi_t[:], ACT.Copy)
        tlo_t = pp.tile([P, NCH], f32)
        nc.vector.tensor_tensor(tlo_t[:], t_t, thi_f[:], op=ALU.subtract)
        tlo_b = pp.tile([P, NCH], bf16)
        nc.vector.tensor_copy(tlo_b[:], tlo_t[:])
        # per-point ceil(t) (exact for trunc or round f32->i32 casts)
        tcp_i = pp.tile([P, NCH], i32)
        nc.vector.tensor_copy(tcp_i[:], t_t)
        tcp_f = pp.tile([P, NCH], f32)
        nc.vector.tensor_copy(tcp_f[:], tcp_i[:])
        tcp_g = pp.tile([P, NCH], f32)
        nc.vector.tensor_tensor(tcp_g[:], t_t, tcp_f[:], op=ALU.is_gt)
        nc.vector.tensor_tensor(tcp_f[:], tcp_f[:], tcp_g[:], op=ALU.add)
        v_b = pp.tile([P, NCH], bf16)
        nc.vector.tensor_copy(v_b[:], v_t)
        f_b = pp.tile([P, NCH], bf16)
        nc.vector.tensor_copy(f_b[:], f_t)

        # ---- channel-routed value planes [P, C, NCH] + rank one-hots ----
        xall = big.tile([P, 5, C, NCH], bf16)
        nc.vector.tensor_tensor(
            xall[:, 2, :, :], crow24[:],
            f_b[:].unsqueeze(1).to_broadcast([P, C, NCH]), op=ALU.is_equal,
        )
        oh3 = xall[:, 2, :, :]
        nc.vector.tensor_tensor(
            xall[:, 0, :, :], oh3,
            thi_t[:].unsqueeze(1).to_broadcast([P, C, NCH]), op=ALU.mult,
        )
        nc.vector.tensor_tensor(
            xall[:, 1, :, :], oh3,
            tlo_b[:].unsqueeze(1).to_broadcast([P, C, NCH]), op=ALU.mult,
        )
        nc.vector.tensor_tensor(
            xall[:, 3, :, :], oh3,
            v_b[:].unsqueeze(1).to_broadcast([P, C, NCH]), op=ALU.mult,
        )
        nc.vector.tensor_tensor(
            xall[:, 4, :, :], oh3,
            tcp_f[:].unsqueeze(1).to_broadcast([P, C, NCH]), op=ALU.mult,
        )

        rkoh = big.tile([P, NSEG, NB, P], bf16)
        rank_b4 = rank_b[:].rearrange("p (s b) -> p s b", s=NSEG)
        grid_p = psum.tile([P, 5, C], f32, tag="grid")
        nmm = 0
        for h in range(2):
            bsl = slice(3 * h, 3 * h + 3)
            nc.vector.tensor_tensor(
                rkoh[:, :, bsl, :],
                rank_b4[:, :, bsl].unsqueeze(3).to_broadcast([P, NSEG, 3, P]),
                irow_b.unsqueeze(1).unsqueeze(1).to_broadcast([P, NSEG, 3, P]),
                op=ALU.is_equal,
            )
            for s in range(NSEG):
                for bk in range(3 * h, 3 * h + 3):
                    ch = NB * s + bk
                    nc.tensor.matmul(
                        grid_p[:], lhsT=rkoh[:, s, bk, :], rhs=xall[:, :, :, ch],
                        start=(nmm == 0), stop=(nmm == NCH - 1),
                    )
                    nmm += 1

        # ---- grid extraction ----
        thi_g = gr.tile([P, C], f32)
        nc.scalar.activation(thi_g[:], grid_p[:, 0, :], ACT.Copy)
        t_g = gr.tile([P, C], f32)
        nc.vector.tensor_tensor(t_g[:], thi_g[:], grid_p[:, 1, :], op=ALU.add)
        occ_g = gr.tile([P, C], f32)
        nc.scalar.activation(occ_g[:], grid_p[:, 2, :], ACT.Copy)
        v_g = gr.tile([P, C], f32)
        nc.scalar.activation(v_g[:], grid_p[:, 3, :], ACT.Copy)
        s_g = gr.tile([P, C], f32)
        nc.vector.tensor_scalar(s_g[:], occ_g[:], BIG, -BIG, ALU.mult, op1=ALU.add)
        nc.vector.tensor_tensor(s_g[:], s_g[:], t_g[:], op=ALU.add)

        # ---- s hi/lo planes, transposed (+negated) via one matmul ----
        s_lo = gr.tile([P, C], f32)
        shiloX = gr.tile([P, P], bf16)
        nc.vector.tensor_copy(shiloX[:, 0:32], s_g[:])
        nc.vector.tensor_tensor(s_lo[:], s_g[:], shiloX[:, 0:32], op=ALU.subtract)
        nc.vector.tensor_copy(shiloX[:, 32:64], s_lo[:])
        nc.vector.tensor_scalar(shiloX[:, 64:96], s_g[:], -1.0, None, ALU.mult)
        nc.vector.tensor_scalar(shiloX[:, 96:128], s_lo[:], -1.0, None, ALU.mult)
        stpX = psum.tile([P, P], f32, tag="tpose")
        nc.tensor.matmul(stpX[:], lhsT=shiloX[:], rhs=id_b, start=True, stop=True)

        # steps[p, c, tau] = (tau >= ceil(t)); ceil came through the grid
        tc_b = gr.tile([P, C], bf16)
        nc.vector.tensor_copy(tc_b[:], grid_p[:, 4, :])
        steps = big.tile([P, C, P], bf16)
        for h in range(2):
            sl = slice(h * 16, h * 16 + 16)
            nc.vector.tensor_tensor(
                steps[:, sl, :],
                irow_b.unsqueeze(1).to_broadcast([P, 16, P]),
                tc_b[:, sl].unsqueeze(2).to_broadcast([P, 16, P]),
                op=ALU.is_ge,
            )

        st2 = gr.tile([64, P], bf16)
        nc.vector.tensor_copy(st2[:], stpX[0:64, :])
        onesneg = gr.tile([P, P], bf16)
        nc.gpsimd.memset(onesneg[0:64, :], 1.0)
        nc.vector.tensor_copy(onesneg[64:128, :], stpX[64:128, :])

        # stX = mask * st2 (in place, top half of rhsAll)
        for h in range(2):
            sl = slice(h * 16, h * 16 + 16)
            nc.vector.tensor_tensor(
                rhs3[0:64, sl, :], rhs3[0:64, sl, :],
                st2[:].unsqueeze(1).to_broadcast([64, 16, P]), op=ALU.mult,
            )

        # ---- all-pairs min + sign-steps + dw + weights + histogram ----
        ivd = gr.tile([P, C], f32)
        dw = gr.tile([P, C], f32)
        w2f = gr.tile([P, C], f32)
        wN = gr.tile([P, 4, C], bf16)
        nc.scalar.activation(wN[:, 0, :], occ_g[:], ACT.Copy)
        hist_p = psum.tile([P, C, 4], f32, tag="hist")
        for g in range(NG):
            gs = slice(g * CG, g * CG + CG)
            sgb = psum.tile([P, CG, P], f32, tag=f"sgb{g % 2}", bufs=2)
            sgb_flat = sgb[:].rearrange("p a b -> p (a b)")
            nc.tensor.matmul(
                sgb_flat, lhsT=onesneg[:],
                rhs=rhs3[:, gs, :].rearrange("p a b -> p (a b)"),
                start=True, stop=False, skip_group_check=True,
            )
            nc.tensor.matmul(
                sgb_flat, lhsT=id_b, rhs=bigeye4,
                start=False, stop=True, skip_group_check=True,
            )
            nc.vector.tensor_reduce(ivd[:, gs], sgb[:], axis=AX.X, op=ALU.min,
                                    apply_absolute_value=True)
            if g % 4 == 3:
                hh = g // 4
                hs = slice(hh * 16, hh * 16 + 16)
                nc.vector.tensor_scalar(dw[:, hs], ivd[:, hs], 2.0**-11, None, ALU.max)
                nc.scalar.activation(dw[:, hs], dw[:, hs], ACT.Sqrt)
                nc.vector.tensor_tensor(w2f[:, hs], occ_g[:, hs], dw[:, hs], op=ALU.mult)
                nc.vector.tensor_copy(wN[:, 1, hs], w2f[:, hs])
                nc.vector.tensor_tensor(wN[:, 2, hs], w2f[:, hs], v_g[:, hs], op=ALU.mult)
                nc.vector.tensor_tensor(wN[:, 3, hs], w2f[:, hs], t_g[:, hs], op=ALU.mult)
                for ch in range(hh * 16, hh * 16 + 16):
                    nc.tensor.matmul(
                        hist_p[:, ch, :], lhsT=steps[:, ch, :],
                        rhs=wN[:, :, ch], start=True, stop=True,
                    )

        # ---- combine (tau on partitions) ----
        cnt_v = hist_p[:, :, 0]
        z_v = hist_p[:, :, 1]
        v_v = hist_p[:, :, 2]
        zt1_v = hist_p[:, :, 3]

        r_t = sb.tile([P, C], f32)
        ce_t = sb.tile([P, C], f32)
        nc.vector.tensor_scalar(r_t[:], z_v, 1e-10, None, ALU.add)
        nc.vector.tensor_scalar(ce_t[:], cnt_v, 1e-10, None, ALU.add)
        nc.vector.tensor_tensor(r_t[:], r_t[:], ce_t[:], op=ALU.mult)
        nc.vector.reciprocal(r_t[:], r_t[:])
        s1_t = sb.tile([P, C], f32)
        nc.vector.tensor_scalar(s1_t[:], zt1_v, imp_c, None, ALU.mult)
        zp_t = sb.tile([P, C], f32)
        nc.vector.tensor_scalar(zp_t[:], z_v, pmp_c, None, ALU.mult)
        nc.vector.tensor_tensor(s1_t[:], s1_t[:], zp_t[:], op=ALU.subtract)

        pack = sb.tile([P, 96], bf16)
        nc.vector.tensor_tensor(pack[:, 0:32], s1_t[:], r_t[:], op=ALU.mult)
        nc.vector.tensor_tensor(pack[:, 32:64], z_v, r_t[:], op=ALU.mult)
        nc.vector.tensor_tensor(pack[:, 64:96], v_v, r_t[:], op=ALU.mult)

        packT_p = psum.tile([96, P], f32, tag="tpose")
        nc.tensor.matmul(packT_p[:], lhsT=pack[:], rhs=id_b, start=True, stop=True)
        packT = sb.tile([96, P], bf16)
        nc.vector.tensor_copy(packT[:], packT_p[:])
        out_p = psum.tile([CO, T], f32, tag="tpose")
        nc.tensor.matmul(out_p[:], lhsT=w3b, rhs=packT[:], start=True, stop=True)
        out_t = sb.tile([CO, T], f32)
        nc.vector.tensor_scalar(out_t[:], out_p[:], blin_c, None, ALU.add)
        nc.sync.dma_start(out_ext[:], out_t[:])

    nc.compile()
    return nc


def _prep_inputs(x, out_positions, W_dist, b_dist, emb, W_vals, b_vals, W_lin, b_lin, kernel_scale):
    import ml_dtypes

    bf = ml_dtypes.bfloat16
    assert abs(float(kernel_scale) - 0.5) < 1e-6  # dw = sqrt(ivd) baked in
    x = np.asarray(x, np.float32)
    pos = np.asarray(out_positions, np.float32)
    max_pos = float(pos.max())
    Wl = np.asarray(W_lin, np.float32).reshape(CO, C, D)
    emb2 = np.asarray(emb, np.float32)[:C] + np.asarray(b_dist, np.float32) + np.asarray(
        b_vals, np.float32
    )
    wd2 = (Wl * np.asarray(W_dist, np.float32)).sum(-1).T
    we2 = np.einsum("ocd,cd->oc", Wl, emb2).T
    wv2 = (Wl * np.asarray(W_vals, np.float32)).sum(-1).T

    q = np.arange(P)
    seg_sel = ((q // 32)[:, None] == np.arange(NSEG)[None, :]).astype(np.float32)
    chm_m = (
        ((q % C)[:, None] == (q % C)[None, :])
        & ((q // C)[:, None] < (q // C)[None, :])
    ).astype(np.float32)

    cst = np.zeros((P, CW), np.float32)

    def put(name, arr, rows=P):
        o, w = _OFF[name]
        cst[0:rows, o : o + w] = arr

    put("segsel", seg_sel)
    put("blin", np.asarray(b_lin, np.float32)[:, None], CO)
    put("imp", np.full((P, 1), 1.0 / max_pos, np.float32))
    put("pmp", (pos / max_pos)[:, None])
    put("iota", (q % 32).astype(np.float32)[:, None])

    cstb = np.zeros((P, CWB), np.float32)

    def putb(name, arr, rows=P):
        o, w = _OFFB[name]
        cstb[0:rows, o : o + w] = arr

    putb("idb", np.eye(P, dtype=np.float32))
    w3 = np.concatenate([wd2, we2, wv2], axis=0)  # (96, CO)
    putb("w3b", w3.astype(np.float32), 96)
    putb("irowb", np.tile(np.arange(P, dtype=np.float32), (P, 1)))
    putb("iotab", (q % 32).astype(np.float32)[:, None])
    cstb = cstb.astype(bf)

    msk = ((q % 32)[0:32, None] == np.arange(C)[None, :]).astype(np.float32)
    mskb = np.ascontiguousarray(
        np.repeat(msk[:, :, None], P, axis=2).reshape(32, C * P).astype(bf)
    )
    xC_a = np.repeat(np.arange(C, dtype=np.float32), NCH)[None, :].astype(bf)

    in_maps = []
    for b in range(B):
        xTb = np.ascontiguousarray(x[b].T)
        xPb = np.ascontiguousarray(
            xTb.reshape(3, NCH, P).transpose(0, 2, 1).reshape(3, N)
        )
        xFb = np.ascontiguousarray(xTb[0:1]).astype(bf)
        in_maps.append(
            {"xF": xFb, "xP": xPb, "cst": cst, "cstb": cstb, "mskb": mskb,
             "xC": np.ascontiguousarray(xC_a), "chm": chm_m}
        )
    return in_maps


def kernel(**inputs) -> np.ndarray:
    from concourse.bass_utils import run_bass_kernel_spmd

    if "nc" not in _cache:
        _cache["nc"] = _build_nc()
    nc = _cache["nc"]

    in_maps = _prep_inputs(**inputs)
    res = run_bass_kernel_spmd(
        nc, in_maps, core_ids=list(range(B)),
        trace=bool(int(os.environ.get("KERNEL_TRACE", "0"))),
    )
    if res.exec_time_ns is not None:
        _cache["exec_time_ns"] = res.exec_time_ns
        _cache["last_result"] = res
    out = np.stack([res.results[i]["out"] for i in range(B)]).astype(np.float32)
    return out


# revision 43
# speedup vs baseline: 1.0697x; 1.0225x over previous
"""Trainium2 Bass kernel for AsyncFeatureExtraction (segment_reduce).

Final design (one batch per core, 8 cores, pure batch data-parallelism):
  * rank[n] = # earlier same-channel points via a segmented cumsum scan
    (4 segments x 32 channels on partitions); cross-segment prefix via a
    chm matmul; ranks extracted straight into chunk-major [128, 24]
    layout by 6 f32 matmuls (lhsT = maskg block, rhs = segsel).
  * grid routing: per 128-point chunk, grid += rkoh.T @ [thi|tlo|occ|v|
    ceil(t)] with bf16-exact hi/lo time planes; one-hots built by DVE
    broadcast compares with all-bf16 operands (2x rate), against
    materialized crow/irow bf16 consts.
  * inv_density: per 4-channel group one bf16 matmul forms s_j - s_i
    from transposed +-[hi|lo] planes ([ones;-st] stationary x
    [mask*st; mask] moving) + a BIG*I eye matmul for the diagonal;
    ivd = min-reduce with apply_absolute_value; dw = sqrt(ivd)
    (kernel_scale is 0.5 for this problem).
  * Z/cnt/V/ZT1 as cumulative step-histograms: steps = (tau >=
    ceil(t)) built bf16-exact (ceil scattered through the grid);
    one [128x128]x[128x4] matmul per channel.
  * combine: R = 1/((Z+eps)(cnt+eps)); out = w3.T @ [S1R;ZR;VR]^T with
    host-prefolded weights; bias add; single output DMA.

Perf notes baked in:
  - all DVE build ops use 16-bit operands with contiguous inner dims
    where possible (any f32 operand halves DVE throughput)
  - DMA bandwidth is shared by all 8 cores: constants are minimized,
    split so early-needed bytes land first, and spread across the
    SP/ACT/Pool queues; the channel mask is one 256KB load plus
    on-chip SBUF->SBUF clones; x is sent twice (n-order for the scan
    broadcast, host-pretransposed chunk-major for the point planes)
  - a PE warmup burst lifts the HAM clock gate during the DMA window
  - PSUM: 8 banks exactly (scratch, grid, tpose, sgb0 x2, sgb1 x2, hist)
"""

import os
import numpy as np

B, N, T, C, D, CO = 8, 3072, 128, 32, 8, 64
P = 128
NCH = N // P          # 24 chunks of 128 consecutive points
NB = 6                # 128-point blocks per segment
NSEG = 4
SEGN = N // NSEG      # 768
NG = 8                # channel groups
CG = C // NG          # 4 channels per group
BIG = 1e10

_cache = {}

# packed f32 const layout (small, DMA'd first so the scan starts early)
_OFF = {}
_cw = 0
for _name, _w in [
    ("segsel", NSEG), ("blin", 1), ("imp", 1), ("pmp", 1), ("iota", 1),
]:
    _OFF[_name] = (_cw, _w)
    _cw += _w
CW = _cw

# packed bf16 const layout
_OFFB = {}
_cwb = 0
for _name, _w in [("idb", P), ("w3b", CO), ("irowb", P), ("iotab", 1)]:
    _OFFB[_name] = (_cwb, _w)
    _cwb += _w
CWB = _cwb


def _build_nc():
    from contextlib import ExitStack

    import concourse.bass as bass
    import concourse.tile as tile
    from concourse import bacc, mybir

    f32 = mybir.dt.float32
    bf16 = mybir.dt.bfloat16
    i32 = mybir.dt.int32
    ALU = mybir.AluOpType
    ACT = mybir.ActivationFunctionType
    AX = mybir.AxisListType

    nc = bacc.Bacc(None)

    xF = nc.declare_dram_parameter("xF", [1, N], bf16, isOutput=False)
    xP = nc.declare_dram_parameter("xP", [3, N], f32, isOutput=False)
    cst = nc.declare_dram_parameter("cst", [P, CW], f32, isOutput=False)
    chm = nc.declare_dram_parameter("chm", [P, P], f32, isOutput=False)
    cstb = nc.declare_dram_parameter("cstb", [P, CWB], bf16, isOutput=False)
    mskb = nc.declare_dram_parameter("mskb", [32, C * P], bf16, isOutput=False)
    xC = nc.declare_dram_parameter("xC", [1, C * NCH], bf16, isOutput=False)
    out_ext = nc.declare_dram_parameter("out", [CO, T], f32, isOutput=True)

    def dram_ap(handle, offset, pattern):
        return bass.AP(handle[:].tensor, offset, pattern)

    with tile.TileContext(nc) as tc, ExitStack() as ctx:
        const = ctx.enter_context(tc.tile_pool(name="const", bufs=1))
        pp = ctx.enter_context(tc.tile_pool(name="perpoint", bufs=1))
        rk = ctx.enter_context(tc.tile_pool(name="rank", bufs=1))
        big = ctx.enter_context(tc.tile_pool(name="big", bufs=1))
        gr = ctx.enter_context(tc.tile_pool(name="grid", bufs=1))
        sb = ctx.enter_context(tc.tile_pool(name="stageD", bufs=1))
        psum = ctx.enter_context(tc.tile_pool(name="psum", bufs=1, space="PSUM"))

        # ---- DMAs (bandwidth is shared by all 8 cores; early = small) ----
        f_seg = rk.tile([P, SEGN], bf16)
        for s in range(NSEG):
            nc.sync.dma_start(
                f_seg[32 * s : 32 * s + 32, :],
                xF[0][SEGN * s : SEGN * (s + 1)][None, :].to_broadcast([32, SEGN]),
            )
        # xP is host-pretransposed so the chunk-major load is contiguous:
        # xP[k, p*NCH + ch] = x[128*ch + p, k]
        pv = pp.tile([P, 3, NCH], f32)
        nc.sync.dma_start(pv[:], dram_ap(xP, 0, [[NCH, P], [N, 3], [1, NCH]]))

        cst_t = const.tile([P, CW], f32)
        nc.scalar.dma_start(cst_t[:], cst[:])
        cstb_t = const.tile([P, CWB], bf16)
        nc.scalar.dma_start(cstb_t[:], cstb[:])
        chm_tt = const.tile([P, P], f32)
        nc.scalar.dma_start(chm_tt[:], chm[:])
        crow24 = big.tile([P, NCH, C], bf16)
        nc.gpsimd.dma_start(
            crow24[:].rearrange("p h c -> p (h c)"),
            xC[0][None, :].to_broadcast([P, C * NCH]),
        )
        rhsAll = big.tile([P, C, P], bf16)
        rhs3 = rhsAll[:]
        msk_src = mskb[:].rearrange("p (c j) -> p c j", c=C)
        nc.gpsimd.dma_start(rhs3[0:32], msk_src)
        nc.gpsimd.dma_start(rhs3[32:64], rhs3[0:32])
        nc.gpsimd.dma_start(rhs3[64:128], rhs3[0:64])

        def cslice(name, rows=P):
            o, w = _OFF[name]
            return cst_t[0:rows, o : o + w]

        def cbslice(name, rows=P):
            o, w = _OFFB[name]
            return cstb_t[0:rows, o : o + w]

        chm_t = chm_tt[:]
        segsel_t = cslice("segsel")
        blin_c = cslice("blin", CO)
        imp_c = cslice("imp")
        pmp_c = cslice("pmp")
        iota_c = cslice("iota")
        id_b = cbslice("idb")
        w3b = cbslice("w3b", 96)
        irow_b = cbslice("irowb")

        bige4 = const.tile([P, CG, P], bf16)
        bigeye4 = bige4[:].rearrange("p a b -> p (a b)")

        # PE warmup as the first tensor work: ~3.5us of sustained matmul
        # activity flips the HAM clock gate to 2.4 GHz before the serial
        # rank matmuls; later PE work keeps it warm.
        warm_p = psum.tile([P, P], f32, tag="tpose")
        for _ in range(36):
            nc.tensor.matmul(warm_p[:], lhsT=id_b, rhs=id_b, start=True, stop=True)

        f_t = pv[:, 0, :]
        v_t = pv[:, 1, :]
        t_t = pv[:, 2, :]

        # ---- ranks: segmented scan + 6 seg-reduce matmuls ----
        oh_seg = rk.tile([P, SEGN], f32)
        nc.vector.tensor_scalar(oh_seg[:], f_seg[:], iota_c, None, ALU.is_equal)
        zseg = rk.tile([P, SEGN], f32)
        nc.gpsimd.memset(zseg[:], 0.0)
        csum = rk.tile([P, SEGN], f32)
        nc.vector.tensor_tensor_scan(
            csum[:], oh_seg[:], zseg[:], 0.0, op0=ALU.add, op1=ALU.add
        )
        totals = rk.tile([P, 1], f32)
        nc.vector.tensor_copy(totals[:], csum[:, SEGN - 1 : SEGN])
        a_p = psum.tile([P, 1], f32, tag="scratch")
        nc.tensor.matmul(a_p[:], lhsT=chm_t, rhs=totals[:], start=True, stop=True)
        a_s = rk.tile([P, 1], f32)
        nc.vector.tensor_scalar(a_s[:], a_p[:], -1.0, None, ALU.add)
        csum2 = rk.tile([P, SEGN], f32)
        maskg = rk.tile([P, SEGN], f32)
        rank_b = pp.tile([P, NCH], bf16)
        # BIG*I eye blocks for the all-pairs diagonal kill (issued after
        # the scan so it doesn't head-block the vector queue)
        nc.vector.tensor_scalar(
            bige4[:], id_b.unsqueeze(1).to_broadcast([P, CG, P]), BIG, None,
            ALU.mult,
        )
        # rank_p[p, s, bk] = rank of point n = 768*s + 128*bk + p; flat
        # (s, bk) order matches pv's contiguous chunk index n//128.
        rank_p = psum.tile([P, NSEG, NB], f32, tag="scratch")
        for h in range(2):
            sl = slice(h * 384, h * 384 + 384)
            nc.vector.tensor_scalar(
                csum2[:, sl], csum[:, sl], a_s[:, 0:1], None, ALU.add
            )
            nc.vector.tensor_tensor(
                maskg[:, sl], csum2[:, sl], oh_seg[:, sl], op=ALU.mult
            )
            for bk in range(3 * h, 3 * h + 3):
                nc.tensor.matmul(
                    rank_p[:, :, bk],
                    lhsT=maskg[:, P * bk : P * bk + P],
                    rhs=segsel_t,
                    start=True,
                    stop=True,
                )
            # ranks < 3072: bf16 rounds >=256 away from 0..127, never to
            # a slot index, so the bf16 compare below stays exact.
            nc.scalar.activation(
                rank_b[:].rearrange("p (s b) -> p s b", s=NSEG)[:, :, 3 * h : 3 * h + 3],
                rank_p[:, :, 3 * h : 3 * h + 3],
                ACT.Copy,
            )

        # ---- per-point planes (bf16 so DVE builds run at 2x) ----
        thi_t = pp.tile([P, NCH], bf16)
        nc.scalar.activation(thi_t[:], t_t, ACT.Copy)
        thi_f = pp.tile([P, NCH], f32)
        nc.scalar.activation(thi_f[:], thi_t[:], ACT.Copy)
        tlo_t = pp.tile([P, NCH], f32)
        nc.vector.tensor_tensor(tlo_t[:], t_t, thi_f[:], op=ALU.subtract)
        tlo_b = pp.tile([P, NCH], bf16)
        nc.vector.tensor_copy(tlo_b[:], tlo_t[:])
        # per-point ceil(t) (exact for trunc or round f32->i32 casts)
        tcp_i = pp.tile([P, NCH], i32)
        nc.vector.tensor_copy(tcp_i[:], t_t)
        tcp_f = pp.tile([P, NCH], f32)
        nc.vector.tensor_copy(tcp_f[:], tcp_i[:])
        tcp_g = pp.tile([P, NCH], f32)
        nc.vector.tensor_tensor(tcp_g[:], t_t, tcp_f[:], op=ALU.is_gt)
        nc.vector.tensor_tensor(tcp_f[:], tcp_f[:], tcp_g[:], op=ALU.add)
        v_b = pp.tile([P, NCH], bf16)
        nc.vector.tensor_copy(v_b[:], v_t)
        f_b = pp.tile([P, NCH], bf16)
        nc.vector.tensor_copy(f_b[:], f_t)

        # ---- channel-routed value planes [P, C, NCH] + rank one-hots ----
        xall = big.tile([P, NCH, 5, C], bf16)
        nc.vector.tensor_tensor(
            xall[:, :, 2, :], crow24[:],
            f_b[:].unsqueeze(2).to_broadcast([P, NCH, C]), op=ALU.is_equal,
        )
        oh3 = xall[:, :, 2, :]
        nc.vector.tensor_tensor(
            xall[:, :, 0, :], oh3,
            thi_t[:].unsqueeze(2).to_broadcast([P, NCH, C]), op=ALU.mult,
        )
        nc.vector.tensor_tensor(
            xall[:, :, 1, :], oh3,
            tlo_b[:].unsqueeze(2).to_broadcast([P, NCH, C]), op=ALU.mult,
        )
        nc.vector.tensor_tensor(
            xall[:, :, 3, :], oh3,
            v_b[:].unsqueeze(2).to_broadcast([P, NCH, C]), op=ALU.mult,
        )
        nc.vector.tensor_tensor(
            xall[:, :, 4, :], oh3,
            tcp_f[:].unsqueeze(2).to_broadcast([P, NCH, C]), op=ALU.mult,
        )

        rkoh = big.tile([P, NSEG, NB, P], bf16)
        rank_b4 = rank_b[:].rearrange("p (s b) -> p s b", s=NSEG)
        grid_p = psum.tile([P, 5, C], f32, tag="grid")
        nmm = 0
        for h in range(2):
            bsl = slice(3 * h, 3 * h + 3)
            nc.vector.tensor_tensor(
                rkoh[:, :, bsl, :],
                rank_b4[:, :, bsl].unsqueeze(3).to_broadcast([P, NSEG, 3, P]),
                irow_b.unsqueeze(1).unsqueeze(1).to_broadcast([P, NSEG, 3, P]),
                op=ALU.is_equal,
            )
            for s in range(NSEG):
                for bk in range(3 * h, 3 * h + 3):
                    ch = NB * s + bk
                    nc.tensor.matmul(
                        grid_p[:], lhsT=rkoh[:, s, bk, :], rhs=xall[:, ch, :, :],
                        start=(nmm == 0), stop=(nmm == NCH - 1),
                    )
                    nmm += 1

        # ---- grid extraction ----
        thi_g = gr.tile([P, C], f32)
        nc.scalar.activation(thi_g[:], grid_p[:, 0, :], ACT.Copy)
        t_g = gr.tile([P, C], f32)
        nc.vector.tensor_tensor(t_g[:], thi_g[:], grid_p[:, 1, :], op=ALU.add)
        occ_g = gr.tile([P, C], f32)
        nc.scalar.activation(occ_g[:], grid_p[:, 2, :], ACT.Copy)
        v_g = gr.tile([P, C], f32)
        nc.scalar.activation(v_g[:], grid_p[:, 3, :], ACT.Copy)
        s_g = gr.tile([P, C], f32)
        nc.vector.tensor_scalar(s_g[:], occ_g[:], BIG, -BIG, ALU.mult, op1=ALU.add)
        nc.vector.tensor_tensor(s_g[:], s_g[:], t_g[:], op=ALU.add)

        # ---- s hi/lo planes, transposed (+negated) via one matmul ----
        s_lo = gr.tile([P, C], f32)
        shiloX = gr.tile([P, P], bf16)
        nc.vector.tensor_copy(shiloX[:, 0:32], s_g[:])
        nc.vector.tensor_tensor(s_lo[:], s_g[:], shiloX[:, 0:32], op=ALU.subtract)
        nc.vector.tensor_copy(shiloX[:, 32:64], s_lo[:])
        nc.vector.tensor_scalar(shiloX[:, 64:96], s_g[:], -1.0, None, ALU.mult)
        nc.vector.tensor_scalar(shiloX[:, 96:128], s_lo[:], -1.0, None, ALU.mult)
        stpX = psum.tile([P, P], f32, tag="tpose")
        nc.tensor.matmul(stpX[:], lhsT=shiloX[:], rhs=id_b, start=True, stop=True)
        # keep the HAM clock gate warm through the DVE-only steps/stX
        # phase so the all-pairs matmuls run at 2.4 GHz
        warm2_p = psum.tile([P, P], f32, tag="tpose")
        for _ in range(10):
            nc.tensor.matmul(warm2_p[:], lhsT=id_b, rhs=id_b, start=True, stop=True)

        # steps[p, c, tau] = (tau >= ceil(t)); ceil came through the grid
        tc_b = gr.tile([P, C], bf16)
        nc.vector.tensor_copy(tc_b[:], grid_p[:, 4, :])
        steps = big.tile([P, C, P], bf16)
        for h in range(2):
            sl = slice(h * 16, h * 16 + 16)
            nc.vector.tensor_tensor(
                steps[:, sl, :],
                irow_b.unsqueeze(1).to_broadcast([P, 16, P]),
                tc_b[:, sl].unsqueeze(2).to_broadcast([P, 16, P]),
                op=ALU.is_ge,
            )

        st2 = gr.tile([64, P], bf16)
        nc.vector.tensor_copy(st2[:], stpX[0:64, :])
        onesneg = gr.tile([P, P], bf16)
        nc.gpsimd.memset(onesneg[0:64, :], 1.0)
        nc.vector.tensor_copy(onesneg[64:128, :], stpX[64:128, :])

        # stX = mask * st2 (in place, top half of rhsAll)
        for h in range(2):
            sl = slice(h * 16, h * 16 + 16)
            nc.vector.tensor_tensor(
                rhs3[0:64, sl, :], rhs3[0:64, sl, :],
                st2[:].unsqueeze(1).to_broadcast([64, 16, P]), op=ALU.mult,
            )

        # ---- all-pairs min + sign-steps + dw + weights + histogram ----
        ivd = gr.tile([P, C], f32)
        dw = gr.tile([P, C], f32)
        w2f = gr.tile([P, C], f32)
        wN = gr.tile([P, 4, C], bf16)
        nc.scalar.activation(wN[:, 0, :], occ_g[:], ACT.Copy)
        hist_p = psum.tile([P, C, 4], f32, tag="hist")
        for g in range(NG):
            gs = slice(g * CG, g * CG + CG)
            sgb = psum.tile([P, CG, P], f32, tag=f"sgb{g % 2}", bufs=2)
            sgb_flat = sgb[:].rearrange("p a b -> p (a b)")
            nc.tensor.matmul(
                sgb_flat, lhsT=onesneg[:],
                rhs=rhs3[:, gs, :].rearrange("p a b -> p (a b)"),
                start=True, stop=False, skip_group_check=True,
            )
            nc.tensor.matmul(
                sgb_flat, lhsT=id_b, rhs=bigeye4,
                start=False, stop=True, skip_group_check=True,
            )
            nc.vector.tensor_reduce(ivd[:, gs], sgb[:], axis=AX.X, op=ALU.min,
                                    apply_absolute_value=True)
            if g % 4 == 3:
                hh = g // 4
                hs = slice(hh * 16, hh * 16 + 16)
                nc.vector.tensor_scalar(dw[:, hs], ivd[:, hs], 2.0**-11, None, ALU.max)
                nc.scalar.activation(dw[:, hs], dw[:, hs], ACT.Sqrt)
                nc.vector.tensor_tensor(w2f[:, hs], occ_g[:, hs], dw[:, hs], op=ALU.mult)
                nc.vector.tensor_copy(wN[:, 1, hs], w2f[:, hs])
                nc.vector.tensor_tensor(wN[:, 2, hs], w2f[:, hs], v_g[:, hs], op=ALU.mult)
                nc.vector.tensor_tensor(wN[:, 3, hs], w2f[:, hs], t_g[:, hs], op=ALU.mult)
                for ch in range(hh * 16, hh * 16 + 16):
                    nc.tensor.matmul(
                        hist_p[:, ch, :], lhsT=steps[:, ch, :],
                        rhs=wN[:, :, ch], start=True, stop=True,
                    )

        # ---- combine (tau on partitions) ----
        cnt_v = hist_p[:, :, 0]
        z_v = hist_p[:, :, 1]
        v_v = hist_p[:, :, 2]
        zt1_v = hist_p[:, :, 3]

        r_t = sb.tile([P, C], f32)
        ce_t = sb.tile([P, C], f32)
        nc.vector.tensor_scalar(r_t[:], z_v, 1e-10, None, ALU.add)
        nc.vector.tensor_scalar(ce_t[:], cnt_v, 1e-10, None, ALU.add)
        nc.vector.tensor_tensor(r_t[:], r_t[:], ce_t[:], op=ALU.mult)
        nc.vector.reciprocal(r_t[:], r_t[:])
        s1_t = sb.tile([P, C], f32)
        nc.vector.tensor_scalar(s1_t[:], zt1_v, imp_c, None, ALU.mult)
        zp_t = sb.tile([P, C], f32)
        nc.vector.tensor_scalar(zp_t[:], z_v, pmp_c, None, ALU.mult)
        nc.vector.tensor_tensor(s1_t[:], s1_t[:], zp_t[:], op=ALU.subtract)

        pack = sb.tile([P, 96], bf16)
        nc.vector.tensor_tensor(pack[:, 0:32], s1_t[:], r_t[:], op=ALU.mult)
        nc.vector.tensor_tensor(pack[:, 32:64], z_v, r_t[:], op=ALU.mult)
        nc.vector.tensor_tensor(pack[:, 64:96], v_v, r_t[:], op=ALU.mult)

        packT_p = psum.tile([96, P], f32, tag="tpose")
        nc.tensor.matmul(packT_p[:], lhsT=pack[:], rhs=id_b, start=True, stop=True)
        packT = sb.tile([96, P], bf16)
        nc.vector.tensor_copy(packT[:], packT_p[:])
        out_p = psum.tile([CO, T], f32, tag="tpose")
        nc.tensor.matmul(out_p[:], lhsT=w3b, rhs=packT[:], start=True, stop=True)
        out_t = sb.tile([CO, T], f32)
        nc.vector.tensor_scalar(out_t[:], out_p[:], blin_c, None, ALU.add)
        nc.sync.dma_start(out_ext[:], out_t[:])

    nc.compile()
    return nc


def _prep_inputs(x, out_positions, W_dist, b_dist, emb, W_vals, b_vals, W_lin, b_lin, kernel_scale):
    import ml_dtypes

    bf = ml_dtypes.bfloat16
    assert abs(float(kernel_scale) - 0.5) < 1e-6  # dw = sqrt(ivd) baked in
    x = np.asarray(x, np.float32)
    pos = np.asarray(out_positions, np.float32)
    max_pos = float(pos.max())
    Wl = np.asarray(W_lin, np.float32).reshape(CO, C, D)
    emb2 = np.asarray(emb, np.float32)[:C] + np.asarray(b_dist, np.float32) + np.asarray(
        b_vals, np.float32
    )
    wd2 = (Wl * np.asarray(W_dist, np.float32)).sum(-1).T
    we2 = np.einsum("ocd,cd->oc", Wl, emb2).T
    wv2 = (Wl * np.asarray(W_vals, np.float32)).sum(-1).T

    q = np.arange(P)
    seg_sel = ((q // 32)[:, None] == np.arange(NSEG)[None, :]).astype(np.float32)
    chm_m = (
        ((q % C)[:, None] == (q % C)[None, :])
        & ((q // C)[:, None] < (q // C)[None, :])
    ).astype(np.float32)

    cst = np.zeros((P, CW), np.float32)

    def put(name, arr, rows=P):
        o, w = _OFF[name]
        cst[0:rows, o : o + w] = arr

    put("segsel", seg_sel)
    put("blin", np.asarray(b_lin, np.float32)[:, None], CO)
    put("imp", np.full((P, 1), 1.0 / max_pos, np.float32))
    put("pmp", (pos / max_pos)[:, None])
    put("iota", (q % 32).astype(np.float32)[:, None])

    cstb = np.zeros((P, CWB), np.float32)

    def putb(name, arr, rows=P):
        o, w = _OFFB[name]
        cstb[0:rows, o : o + w] = arr

    putb("idb", np.eye(P, dtype=np.float32))
    w3 = np.concatenate([wd2, we2, wv2], axis=0)  # (96, CO)
    putb("w3b", w3.astype(np.float32), 96)
    putb("irowb", np.tile(np.arange(P, dtype=np.float32), (P, 1)))
    putb("iotab", (q % 32).astype(np.float32)[:, None])
    cstb = cstb.astype(bf)

    msk = ((q % 32)[0:32, None] == np.arange(C)[None, :]).astype(np.float32)
    mskb = np.ascontiguousarray(
        np.repeat(msk[:, :, None], P, axis=2).reshape(32, C * P).astype(bf)
    )
    xC_a = np.tile(np.arange(C, dtype=np.float32), NCH)[None, :].astype(bf)

    in_maps = []
    for b in range(B):
        xTb = np.ascontiguousarray(x[b].T)
        xPb = np.ascontiguousarray(
            xTb.reshape(3, NCH, P).transpose(0, 2, 1).reshape(3, N)
        )
        xFb = np.ascontiguousarray(xTb[0:1]).astype(bf)
        in_maps.append(
            {"xF": xFb, "xP": xPb, "cst": cst, "cstb": cstb, "mskb": mskb,
             "xC": np.ascontiguousarray(xC_a), "chm": chm_m}
        )
    return in_maps


def kernel(**inputs) -> np.ndarray:
    from concourse.bass_utils import run_bass_kernel_spmd

    if "nc" not in _cache:
        _cache["nc"] = _build_nc()
    nc = _cache["nc"]

    in_maps = _prep_inputs(**inputs)
    res = run_bass_kernel_spmd(
        nc, in_maps, core_ids=list(range(B)),
        trace=bool(int(os.environ.get("KERNEL_TRACE", "0"))),
    )
    if res.exec_time_ns is not None:
        _cache["exec_time_ns"] = res.exec_time_ns
        _cache["last_result"] = res
    out = np.stack([res.results[i]["out"] for i in range(B)]).astype(np.float32)
    return out
